# revision 37
# baseline (speedup 1.0000x reference)
"""Trainium2 Bass kernel for nn_CausalSelfAttentionSynapse (v12).

Math (per reference):
    qk = g @ W_lift.T; q,k heads of dim D=64
    lse[b,h,t] = logsumexp_{j<=t} (q_t . k_j)
    out[b,t]  = sum_h lse[b,h,t] * w[h],  w[h] = sum_g W_proj[g,h]

Sharding: 8 cores = 4 batches x 2 head-groups (8 heads each).

Per-core design.  The kernel is jointly limited by (a) the PE, which
is PSUM-write-port bound on the score materialization (the K=64
contraction streams one 128-high fp32 column per cycle no matter how
the matmuls are tiled) plus the lift MACs, and (b) the exp+rowsum
consumers.  The consumer work is therefore split between ScalarE and
VectorE so neither becomes the critical path:
  - Host pre-transposes g[b] and the head-group's W rows into e-major
    bf16 layouts (gT, wT) -> no on-device transposes at all.  Input
    DMAs are split across the HWDGE (sync) and SWDGE (gpsimd) queues
    so the first lift starts ~7us in.
  - Lift: q/k per head pair via PE matmuls (bf16 in, fp32 PSUM), then
    DVE tensor_copy to SBUF bf16.  Lift chunks for the next pair ride
    inside the current pair's qi 4..11 window.
  - Scores: per (pair, q-tile of 128 rows, head) causal row block of
    kneed=128*(qi+1) columns, produced head-after-head so consumption
    of head 0 overlaps production of head 1 (a both-heads group can
    pin all 8 PSUM banks and force PE<->consumer alternation).  The
    causal mask of the diagonal block is ADDED by one extra matmul
    (lhsT=I, rhs=strictly-upper -30000); whole-DVE regions (kneed<=512)
    skip that matmul and fold the mask into the Schraudolph bias tile
    instead (saturating the f32->i32 convert to -0.0f).
  - exp+rowsum per the LEAD table: the leading LEAD[qi] columns of
    each region go to the DVE as a 2-pass Schraudolph exp
    (tensor_scalar mult/add with f32->i32 round-to-nearest convert ->
    i32 stage holding fp32 exp bits; masked -30000 scores saturate the
    convert to INT32_MIN = -0.0f, a no-op in the sum; pass2 re-reads
    the stage bitcast f32 with accum_out).  The tail (incl. diagonal)
    goes to ScalarE as exp-in-place on PSUM with accum_out.  LEAD is
    tuned so ACT ~= DVE busy time and both engines get work in every
    qi phase.
  - Finale: lse ~= (bits(sumsA+sumsB) - BLN)*ln2/2^23 (Schraudolph
    log, no ACT table switch); the ln2/2^23 factor and the per-head
    weights are folded into one replicated weight tile; 3 folding adds
    reduce 8 heads; DMA out.
Accuracy: bf16 matmuls + Schraudolph exp/log land at rel err ~4.7e-3
vs the fp64 reference (gate is 2e-2).
"""

import numpy as np
import ml_dtypes

B, T, E, H = 4, 2048, 1024, 16
D = 64
NCORES = 8
NE = 8         # 128-row chunks of e
PAIRS = 4      # head pairs per core
MASKV = -30000.0

# Schraudolph constants (int32 / f32-bit variant), round-to-nearest.
A32 = float(2.0**23 / np.log(2.0))
B32 = float(127.0 * 2.0**23 - 366000.0)

# Schraudolph log constants for the finale (lse = (bits(s)-BLN)*KLN)
BLN_I = int(127 * 2**23 - 366000)
KLN = float(np.log(2.0) / 2.0**23)

# DVE lead columns per q-tile (rest of the region, incl. the diagonal
# block, is consumed by ScalarE).  L == kneed => whole region on DVE.
LEAD = [128, 256, 384, 0, 0, 0, 256, 0, 256, 256, 384, 0,
        1024, 256, 1024, 896]
# q-tiles whose pass2 reduce runs on GPSIMD instead of the DVE
GPS_QI = set()

_CACHE = {}


def _build():
    import concourse.bass as bass  # noqa: F401
    import concourse.tile as tile
    from concourse import bacc, mybir

    f32 = mybir.dt.float32
    bf16 = mybir.dt.bfloat16
    i32 = mybir.dt.int32
    EXP = mybir.ActivationFunctionType.Exp
    LN = mybir.ActivationFunctionType.Ln
    AX = mybir.AxisListType.X
    MUL = mybir.AluOpType.mult
    ADD = mybir.AluOpType.add
    SHR = mybir.AluOpType.logical_shift_right
    SUB = mybir.AluOpType.subtract
    AND = mybir.AluOpType.bitwise_and
    OR = mybir.AluOpType.bitwise_or
    LN2 = float(np.log(2.0))

    nc = bacc.Bacc("TRN2", target_bir_lowering=False, debug=False,
                   num_devices=NCORES)

    # gt rows: half*1024 + e*128 + p ; cols: t-within-half
    g_d = nc.dram_tensor("gt", [T, E], bf16, kind="ExternalInput").ap()
    # wt rows: pair*1024 + e*128 + p ; cols: f-within-pair (q0 q1 k0 k1)
    w_d = nc.dram_tensor("wt", [4 * E, 256], bf16, kind="ExternalInput").ap()
    tri_d = nc.dram_tensor("tri", [128, 128], bf16, kind="ExternalInput").ap()
    id_d = nc.dram_tensor("identb", [128, 128], bf16,
                          kind="ExternalInput").ap()
    wr_d = nc.dram_tensor("wrep", [128, 128], f32, kind="ExternalInput").ap()
    mb_d = nc.dram_tensor("maskb", [128, 512], f32,
                          kind="ExternalInput").ap()
    out_d = nc.dram_tensor("out_part", [128, 16], f32,
                           kind="ExternalOutput").ap()

    with tile.TileContext(nc) as tc:
        with (
            tc.tile_pool(name="consts", bufs=1) as consts,
            tc.tile_pool(name="big", bufs=1) as big,
            tc.tile_pool(name="qkp", bufs=3) as qkp,
            tc.tile_pool(name="stg", bufs=3) as stg,
            tc.tile_pool(name="sums", bufs=1) as sums,
            tc.tile_pool(name="fin", bufs=1) as fin,
            tc.tile_pool(name="ps", bufs=1, space="PSUM") as ps,
        ):
            # ---- constants (first: tiny, unblock warm-up) -----------------
            tri = consts.tile([128, 128], bf16, name="tri", tag="tri")
            nc.sync.dma_start(out=tri[:], in_=tri_d[:])
            identb = consts.tile([128, 128], bf16, name="identb", tag="id")
            nc.sync.dma_start(out=identb[:], in_=id_d[:])
            wrep = consts.tile([128, 128], f32, name="wrep", tag="wrep")
            nc.sync.dma_start(out=wrep[:], in_=wr_d[:])
            maskb = consts.tile([128, 512], f32, name="maskb", tag="maskb")
            nc.sync.dma_start(out=maskb[:], in_=mb_d[:])

            # ---- big SBUF layouts (host-pretransposed, bf16) --------------
            # gT col = e*2048 + t ; wT col = e*1024 + pair*256 + fw
            gT = big.tile([128, NE * T], bf16, name="gT", tag="gT")
            wT = big.tile([128, NE * E], bf16, name="wT", tag="wT")

            def dma_wt(pr):
                for e in range(NE):
                    nc.sync.dma_start(
                        out=wT[:, e * 1024 + pr * 256: e * 1024 + pr * 256 + 256],
                        in_=w_d[pr * 1024 + e * 128: pr * 1024 + e * 128 + 128, :])

            def dma_gt(tcn):
                # global t-chunk tcn in 0..3 -> (half, tc-within-half)
                half, tch = divmod(tcn, 2)
                for e in range(NE):
                    nc.sync.dma_start(
                        out=gT[:, e * 2048 + tcn * 512: e * 2048 + tcn * 512 + 512],
                        in_=g_d[half * 1024 + e * 128: half * 1024 + e * 128 + 128,
                                tch * 512: tch * 512 + 512])

            # critical-path DMAs first (gt1 goes on the SWDGE queue so
            # it lands in parallel with gt0)
            dma_wt(0)
            dma_gt(0)

            # ---- PSUM ring (8 banks of 512 fp32, bank-aligned regions) ----
            ring = ps.tile([128, 4096], f32, name="ring", tag="ring")
            ring_pos = [0]

            def ring_alloc(nbanks):
                if ring_pos[0] + nbanks > 8:
                    ring_pos[0] = 0
                off = ring_pos[0] * 512
                ring_pos[0] += nbanks
                return off

            # PE warm-up junk matmuls (from a memset tile, no DMA
            # dependency) so HAM is at 8/8 when the real lift starts; a
            # dummy exp preloads the ACT table set.
            wub = fin.tile([128, 128], bf16, name="wub", tag="wub")
            nc.vector.memset(wub[:], 1.0)
            wz = fin.tile([128, 1], f32, name="wz", tag="wz")
            nc.vector.memset(wz[:], 0.0)
            joff = ring_alloc(1)
            for _ in range(48):
                nc.tensor.matmul(ring[:, joff:joff + 128], lhsT=wub[:],
                                 rhs=wub[:], start=True, stop=True)
            jx = fin.tile([128, 1], f32, name="jx", tag="jx")
            nc.scalar.activation(jx[:], wz[:], EXP)

            # sums tiles (ACT accumulator results + DVE partial sums)
            sumsA = sums.tile([128, 128], f32, name="sumsA", tag="sumsA")
            nc.vector.memset(sumsA[:], 0.0)
            sumsB = sums.tile([128, 128], f32, name="sumsB", tag="sumsB")
            nc.vector.memset(sumsB[:], 0.0)
            junk = sums.tile([128, 2048], f32, name="junk", tag="junk")

            # rest of the input DMAs, alternating queues: sync carries
            # gt2 after gt0; the SWDGE queue carries gt1/gt3 + the
            # remaining W tiles in parallel
            def dma_gt2(tcn):
                half, tch = divmod(tcn, 2)
                for e in range(NE):
                    nc.gpsimd.dma_start(
                        out=gT[:, e * 2048 + tcn * 512: e * 2048 + tcn * 512 + 512],
                        in_=g_d[half * 1024 + e * 128: half * 1024 + e * 128 + 128,
                                tch * 512: tch * 512 + 512])

            dma_gt2(1)
            dma_gt(2)
            dma_gt2(3)
            for pr in range(1, PAIRS):
                for e in range(NE):
                    nc.gpsimd.dma_start(
                        out=wT[:, e * 1024 + pr * 256: e * 1024 + pr * 256 + 256],
                        in_=w_d[pr * 1024 + e * 128: pr * 1024 + e * 128 + 128, :])

            qkts = {}
            copy_ctr = [0]

            def lift_chunk(pr, ft, tcn):
                """qkt[pr] cols [ft*2048 + tcn*512, +512) from W f-tile."""
                off = ring_alloc(1)
                pt = ring[:, off:off + 512]
                w0 = pr * 256 + ft * 128
                for e in range(NE):
                    nc.tensor.matmul(
                        pt,
                        lhsT=wT[:, e * 1024 + w0: e * 1024 + w0 + 128],
                        rhs=gT[:, e * 2048 + tcn * 512:
                               e * 2048 + tcn * 512 + 512],
                        start=(e == 0), stop=(e == NE - 1))
                dst = qkts[pr][:, ft * 2048 + tcn * 512:
                               ft * 2048 + tcn * 512 + 512]
                nc.vector.tensor_copy(dst, pt)
                copy_ctr[0] += 1

            def score_qtile(pr, qi):
                """Causal scores + exp/rowsum for both heads of pair pr,
                query rows [qi*128, +128).  Heads are produced one after
                the other so consumption of head 0 overlaps production of
                head 1 (a full-qi group would pin up to all 8 PSUM banks
                and force strict PE<->consumer alternation)."""
                kneed = 128 * (qi + 1)
                nb = (kneed + 511) // 512
                L = min(LEAD[qi], kneed)
                tail = kneed - L
                qkt = qkts[pr]
                stage = None
                if L > 0:
                    stage = stg.tile([128, 4096], i32, name="stage",
                                     tag="stage")
                whole_dve = (L == kneed and kneed <= 512)
                p2s = []
                for h in range(2):
                    off = ring_alloc(nb)
                    lhsT = qkt[64 * h:64 * h + 64, qi * 128: qi * 128 + 128]
                    a = 0
                    while a < kneed:
                        sz = min(512, kneed - a)
                        last = (a + sz == kneed)
                        nc.tensor.matmul(
                            ring[:, off + a: off + a + sz],
                            lhsT=lhsT,
                            rhs=qkt[64 * h:64 * h + 64,
                                    2048 + a: 2048 + a + sz],
                            start=True,
                            stop=True if (whole_dve and last) else not last)
                        a += sz
                    if not whole_dve:
                        # additive causal mask on the diagonal block
                        nc.tensor.matmul(
                            ring[:, off + kneed - 128: off + kneed],
                            lhsT=identb[:], rhs=tri[:], start=False,
                            stop=True)
                    col = (2 * pr + h) * 16 + qi
                    if L > 0:
                        if whole_dve:
                            # causal mask via bias tile: masked lanes
                            # saturate the f32->i32 convert to INT32_MIN
                            # = -0.0f, a no-op in the downstream sum
                            nc.vector.scalar_tensor_tensor(
                                out=stage[:, h * L: h * L + L],
                                in0=ring[:, off: off + L],
                                scalar=A32,
                                in1=maskb[:, 512 - kneed: 512],
                                op0=MUL, op1=ADD)
                        else:
                            nc.vector.tensor_scalar(
                                out=stage[:, h * L: h * L + L],
                                in0=ring[:, off: off + L],
                                scalar1=A32, scalar2=B32, op0=MUL, op1=ADD)
                        p2s.append((h, col))
                    if tail > 0:
                        reg = ring[:, off + L: off + kneed]
                        nc.scalar.activation(
                            reg, reg, EXP,
                            accum_out=sumsA[:, col:col + 1])
                for h, col in p2s:
                    nc.vector.tensor_scalar(
                        out=junk[:, 0:L],
                        in0=stage[:, h * L: h * L + L].bitcast(f32),
                        scalar1=1.0, scalar2=0.0, op0=MUL, op1=ADD,
                        accum_out=sumsB[:, col:col + 1])

            # ---- schedule -------------------------------------------------
            # Pair 0 lifts itself early (spread over qi0-5); pair p+1's
            # lift chunks ride uniformly in pair p's qi 8..15 window, so
            # the PE always has dense work during the consumer-heavy
            # high-qi phase (keeps HAM at 8/8).
            qkts[0] = qkp.tile([128, 2 * T], bf16, name="qkt0", tag="qkt")
            lift_chunk(0, 0, 0)
            lift_chunk(0, 1, 0)
            p0_self = {0: [(0, 0, 1), (0, 1, 1)], 1: [(0, 0, 2)],
                       2: [(0, 1, 2)], 3: [(0, 0, 3)], 4: [(0, 1, 3)]}
            p0_next = {5: [(1, 0, 0), (1, 1, 0)], 6: [(1, 0, 1)],
                       7: [(1, 1, 1)], 8: [(1, 0, 2)], 9: [(1, 1, 2)],
                       10: [(1, 0, 3)], 11: [(1, 1, 3)]}
            order = [(0, 0), (1, 0), (0, 1), (1, 1),
                     (0, 2), (1, 2), (0, 3), (1, 3)]
            qkts[1] = qkp.tile([128, 2 * T], bf16, name="qkt1", tag="qkt")
            for qi in range(16):
                score_qtile(0, qi)
                for ch in p0_self.get(qi, []):
                    lift_chunk(*ch)
                for ch in p0_next.get(qi, []):
                    lift_chunk(*ch)

            for pr in range(1, PAIRS):
                nxt = pr + 1
                if nxt < PAIRS:
                    qkts[nxt] = qkp.tile([128, 2 * T], bf16,
                                         name=f"qkt{nxt}", tag="qkt")
                for qi in range(16):
                    score_qtile(pr, qi)
                    if nxt < PAIRS and 6 <= qi < 14:
                        ft, tcn = order[qi - 6]
                        lift_chunk(nxt, ft, tcn)

            # ---- finale: lse ~= (bits(sumsA+sumsB) - BLN)*ln2/2^23 --------
            # (Schraudolph log; the ln2/2^23 factor is folded into wrep
            # host-side.)  out = sum_h w[h]*lse_h.
            total = fin.tile([128, 128], f32, name="total", tag="total")
            nc.vector.tensor_add(total[:], sumsA[:], sumsB[:])
            lse = fin.tile([128, 128], f32, name="lse", tag="lse")
            nc.vector.tensor_scalar(out=lse[:], in0=total[:].bitcast(i32),
                                    scalar1=BLN_I, scalar2=None, op0=SUB)
            wl = fin.tile([128, 128], f32, name="wl", tag="wl")
            nc.vector.tensor_mul(wl[:], lse[:], wrep[:])
            h64 = fin.tile([128, 64], f32, name="h64", tag="h64")
            nc.vector.tensor_add(h64[:], wl[:, 0:64], wl[:, 64:128])
            h32 = fin.tile([128, 32], f32, name="h32", tag="h32")
            nc.vector.tensor_add(h32[:], h64[:, 0:32], h64[:, 32:64])
            facc = fin.tile([128, 16], f32, name="facc", tag="facc")
            nc.vector.tensor_add(facc[:], h32[:, 0:16], h32[:, 16:32])
            nc.sync.dma_start(out=out_d[:], in_=facc[:])

    nc.compile()
    return nc


def _get_nc():
    if "nc" not in _CACHE:
        _CACHE["nc"] = _build()
    return _CACHE["nc"]


def kernel(g, W_lift, W_proj):
    from concourse.bass_utils import run_bass_kernel_spmd

    bf16 = ml_dtypes.bfloat16
    g = np.asarray(g, dtype=np.float32)
    W_lift = np.asarray(W_lift, dtype=np.float32)
    W_proj = np.asarray(W_proj, dtype=np.float32)

    nc = _get_nc()
    w = W_proj.sum(axis=0).astype(np.float32)          # w[h] = sum_g W_proj[g,h]
    tri = np.triu(np.full((128, 128), MASKV, np.float32), k=1).astype(bf16)
    identb = np.eye(128, dtype=np.float32).astype(bf16)
    maskb = np.full((128, 512), B32, np.float32)
    maskb[:, 384:] += np.triu(np.full((128, 128), -4e11, np.float32), k=1)

    in_maps = []
    for core in range(NCORES):
        b, hg = core // 2, core % 2
        rows = []
        for p in range(PAIRS):
            h0 = hg * 8 + 2 * p
            h1 = h0 + 1
            rows += list(range(h0 * D, h0 * D + D))
            rows += list(range(h1 * D, h1 * D + D))
            rows += list(range(E + h0 * D, E + h0 * D + D))
            rows += list(range(E + h1 * D, E + h1 * D + D))
        W_slice = W_lift[rows, :]                      # [1024 f, 1024 e]
        wt_host = np.ascontiguousarray(
            W_slice.T.reshape(NE, 128, PAIRS, 256)
            .transpose(2, 0, 1, 3).reshape(4 * E, 256)).astype(bf16)
        gt_host = np.ascontiguousarray(
            g[b].reshape(2, 1024, NE, 128)
            .transpose(0, 2, 3, 1).reshape(T, E)).astype(bf16)
        wrep = np.ascontiguousarray(
            np.broadcast_to(np.repeat(w[hg * 8: hg * 8 + 8], 16)[None, :]
                            * KLN, (128, 128))).astype(np.float32)
        in_maps.append({
            "gt": gt_host,
            "wt": wt_host,
            "tri": tri,
            "identb": identb,
            "wrep": wrep,
            "maskb": maskb,
        })

    res = run_bass_kernel_spmd(nc, in_maps, core_ids=list(range(NCORES)))
    _CACHE["last_results"] = res
    _CACHE["last_in_maps"] = in_maps

    out = np.zeros((B, T), dtype=np.float32)
    for core in range(NCORES):
        b = core // 2
        part = res.results[core]["out_part"]           # (128, 16)
        out[b] += part.T.reshape(-1)
    return out
